# revision 23
# baseline (speedup 1.0000x reference)
"""Varlen causal attention (flash_attn_varlen semantics) on 8 Trainium2 cores.

Sharding: 16 heads across 8 cores (2 heads/core, Ulysses-style head shard,
identity comms). Each core runs the same SPMD Bass program on its head slice.

Key design (transpose-free inner loop): compute S^T = K @ Q^T instead of
S = Q @ K^T.  Then P^T = exp(S^T * scale) comes out of the activation engine
already in [k, q] layout, which is exactly the stationary-operand layout the
PV matmul needs (lhsT = P^T chunk, rhs = V block) -- no P transposes at all.

Per head:
  prep: load Q,K,V (both heads fused per DMA, 1KB contiguous elements, Q/K
        issued before V); PE-transpose Q,K into [D, L] bf16; V + ones col.
  main: for each 256-row q superblock, for each in-mask k block j:
        S^T tile = K_j @ Q^T  (bf16, PSUM f32), exp on ScalarE (bf16 out),
        causal/segment masking on GpSimd, then PV matmuls accumulate
        O[q, 0:130] per 128-q chunk over j (col 0 = softmax denominator from
        a ones column in V).  Finalize: reciprocal + scale on DVE, DMA out.
The (I, j) tile list, trimmed to the causal x segment block mask, is
specialized on the host from cu_seqlens at trace time.  Width-2 tiles are
packed first so no S^T matmul straddles a PSUM bank.
"""

import numpy as np

L = 4096
H = 16
D = 128
N_CORES = 8
H_PER_CORE = H // N_CORES
SCALE = 1.0 / float(np.sqrt(D))
QB = 128          # q/k block size
SB = 4            # q blocks per superblock (512 q rows)
GROUP_UNITS = 8   # 128-col units per S^T PSUM group tile ([128,1024] f32)
BANK_UNITS = 4    # 128-col units per PSUM bank


def _seg_starts(cu: np.ndarray) -> np.ndarray:
    """Per-token segment start, exactly mirroring the reference searchsorted."""
    tok = np.arange(L)
    seg = np.searchsorted(cu[1:-1], tok, side="right")
    starts = np.concatenate([[0], cu[1:-1]])
    return starts[seg]


def _build_plan(cu: np.ndarray):
    """Host-side specialization of the block-sparse attention pattern.

    Returns a list (one entry per superblock I) of dicts:
      groups: list of groups; each group has
              runs:  [(u, j, i, n)]  one S^T matmul per run (n units wide)
              units: [(u, j, i)]     per-128-col bookkeeping
      masks:  list of ("tri"|"rows"|"zero", group_idx, unit_off, *args)
      pv:     {chunk i: [(group_idx, unit_off, j), ...]}
    """
    ss = _seg_starts(cu)
    n_qb = L // QB
    k_lo_b = [int(ss[i * QB]) // QB for i in range(n_qb)]
    bounds = [int(b) for b in cu[1:-1] if 0 < int(b) < L]

    plan = []
    for I in range(n_qb // SB):
        i0, i1 = SB * I, SB * I + SB - 1
        tiles = []
        for j in range(k_lo_b[i0], i1 + 1):
            qsb = max(i0, j)
            qeb = qsb
            for i in range(qsb, i1 + 1):
                if k_lo_b[i] <= j:
                    qeb = i + 1
                else:
                    break
            if qeb > qsb:
                tiles.append((j, qsb, qeb - qsb))
        # first-fit-decreasing into PSUM banks: each tile becomes exactly one
        # matmul (never straddles a bank); unfilled bank tails are holes that
        # get exp'd as garbage and never read downstream
        tiles.sort(key=lambda t: (-t[2], t[0]))

        groups = []
        masks = []
        pv = {i: [] for i in range(i0, i1 + 1)}

        def place(w):
            if groups:
                g = groups[-1]
                for b in range(GROUP_UNITS // BANK_UNITS):
                    if BANK_UNITS - g["banks"][b] >= w:
                        u = b * BANK_UNITS + g["banks"][b]
                        g["banks"][b] += w
                        return len(groups) - 1, u
            groups.append({"runs": [], "units": [],
                           "banks": [0] * (GROUP_UNITS // BANK_UNITS)})
            groups[-1]["banks"][0] = w
            return len(groups) - 1, 0

        for (j, qsb, w) in tiles:
            g, u = place(w)
            groups[g]["runs"].append((u, j, qsb, w))
            for c in range(w):
                ii = qsb + c
                uu = u + c
                groups[g]["units"].append((uu, j, ii))
                pv[ii].append((g, uu, j))
                if ii == j:
                    masks.append(("tri", g, uu))
                q0u = ii * QB
                for b in bounds:
                    if j * QB < b < (j + 1) * QB:
                        c0 = max(0, b - q0u)
                        rb = b - j * QB
                        if c0 < QB:
                            masks.append(("rows", g, uu, c0, rb))
                    elif (j + 1) * QB <= b:
                        c0 = b - q0u
                        if 0 <= c0 < QB:
                            masks.append(("zero", g, uu, c0))
        for g in groups:
            nb = GROUP_UNITS // BANK_UNITS
            last = max(b for b in range(nb) if g["banks"][b]) if any(
                g["banks"]) else 0
            g["gw_units"] = last * BANK_UNITS + g["banks"][last]
        # PV accumulation order per chunk must be deterministic; sort by j so
        # start/stop flags are simply first/last of the list.
        for i in pv:
            pv[i].sort(key=lambda t: t[2])
        plan.append({"groups": groups, "masks": masks, "pv": pv,
                     "i0": i0, "n_chunks": i1 - i0 + 1})
    return plan


def _build(cu: np.ndarray):
    import concourse.mybir as mybir
    import concourse.tile as tile
    from concourse import bacc
    from concourse.masks import make_identity

    f32 = mybir.dt.float32
    bf16 = mybir.dt.bfloat16
    AF = mybir.ActivationFunctionType
    n_qb = L // QB
    plan = _build_plan(cu)

    nc = bacc.Bacc("TRN2", target_bir_lowering=False, debug=False,
                   num_devices=N_CORES)
    q_d = nc.dram_tensor("q", [L, H_PER_CORE, D], f32, kind="ExternalInput")
    k_d = nc.dram_tensor("k", [L, H_PER_CORE, D], f32, kind="ExternalInput")
    v_d = nc.dram_tensor("v", [L, H_PER_CORE, D], f32, kind="ExternalInput")
    o_d = nc.dram_tensor("out", [L, H_PER_CORE, D], f32, kind="ExternalOutput")

    with tile.TileContext(nc) as tc:
        with (
            tc.tile_pool(name="consts", bufs=1) as consts,
            tc.tile_pool(name="stage", bufs=1) as stage,
            tc.tile_pool(name="big", bufs=1) as big,
            tc.tile_pool(name="psb", bufs=18) as psb,
            tc.tile_pool(name="osb", bufs=2) as osb,
            tc.tile_pool(name="rsb", bufs=4) as rsb,
            tc.tile_pool(name="s_ps", bufs=2, space="PSUM") as s_ps,
            tc.tile_pool(name="o_ps", bufs=1, space="PSUM") as o_ps,
        ):
            ident = consts.tile([128, 128], f32)
            make_identity(nc, ident[:])

            # ---- DMA loads: both heads fused per span (contiguous 1KB per
            # (p, t) element => best HBM efficiency), early spans first ----
            qs = stage.tile([128, n_qb, H_PER_CORE, D], f32, tag="qs")
            ks = stage.tile([128, n_qb, H_PER_CORE, D], f32, tag="ks")
            vs = stage.tile([128, n_qb, H_PER_CORE, D], f32, tag="vs")
            for b0 in range(0, n_qb, 8):
                r = slice(b0 * QB, (b0 + 8) * QB)
                for t_d, t_s in ((qs, q_d), (ks, k_d), (vs, v_d)):
                    nc.sync.dma_start(
                        t_d[:, b0:b0 + 8, :, :],
                        t_s[r, :, :].rearrange("(t p) h d -> p t h d", p=128))

            # per-head prep state; transposes + V casts are emitted on demand
            # inside the main loop so compute tracks DMA arrival
            hstate = []
            for h in range(H_PER_CORE):
                vA = big.tile([128, n_qb, 130], bf16, tag=f"vA{h}")
                nc.gpsimd.memset(vA[:, :, 0:1], 1.0)
                qT = big.tile([128, L], bf16, tag=f"qT{h}")
                kT = big.tile([128, L], bf16, tag=f"kT{h}")
                hstate.append({"vA": vA, "qT": qT, "kT": kT, "done": 0,
                               "pending": None,
                               "ost": {"tile": None, "i0": 0, "filled": 0}})

            def emit_prep(h, need_b):
                hs = hstate[h]
                while hs["done"] < min(need_b, n_qb):
                    b0 = hs["done"]
                    nb = min(8, n_qb - b0)
                    for srct, dstT in ((qs, hs["qT"]), (ks, hs["kT"])):
                        trp = s_ps.tile([128, 1024], f32, tag="s", name="trp")
                        for t in range(nb):
                            nc.tensor.transpose(
                                trp[:, t * QB:(t + 1) * QB],
                                srct[:, b0 + t, h, :], ident[:])
                        nc.vector.tensor_copy(
                            dstT[:, b0 * QB:(b0 + nb) * QB],
                            trp[:, 0:nb * QB])
                    nc.vector.tensor_copy(hs["vA"][:, b0:b0 + nb, 1:129],
                                          vs[:, b0:b0 + nb, h, :])
                    hs["done"] += nb

            def flush_out(h):
                st = hstate[h]["ost"]
                nf = st["filled"]
                if not nf:
                    return
                i0 = st["i0"]
                nc.sync.dma_start(
                    o_d[i0 * QB:(i0 + nf) * QB, h, :].rearrange(
                        "(t p) d -> p t d", p=128),
                    st["tile"][:, 0:nf, :])
                st["tile"] = None
                st["filled"] = 0

            def emit_pv_finalize(h, pend):
                # O chunk c lives at PSUM bank c//2, slot (c%2)*130; the two
                # accumulation groups per bank run sequentially (chunk-major)
                # as required by the one-group-per-bank rule.
                I, ptiles = pend
                sbp = plan[I]
                i0 = sbp["i0"]
                vA = hstate[h]["vA"]
                st = hstate[h]["ost"]
                nch = sbp["n_chunks"]
                slot = lambda c: (c // 2) * 512 + (c % 2) * 130
                o_t = o_ps.tile([128, 1024], f32, tag=f"o{h}", name="o_t")
                for c in range(nch):
                    i = i0 + c
                    lst = sbp["pv"][i]
                    for nn, (g, u, j) in enumerate(lst):
                        nc.tensor.matmul(
                            o_t[:, slot(c):slot(c) + 130],
                            ptiles[g][:, u * QB:(u + 1) * QB],
                            vA[:, j, 0:130],
                            start=(nn == 0), stop=(nn == len(lst) - 1))
                rec = rsb.tile([128, SB, 1], f32, tag="r")
                for b in range((nch + 1) // 2):
                    den = o_t[:, b * 512:b * 512 + 260].rearrange(
                        "p (c x) -> p c x", c=2)
                    nc.vector.reciprocal(rec[:, 2 * b:2 * b + 2, :],
                                         den[:, :, 0:1])
                if st["tile"] is None:
                    st["tile"] = osb.tile([128, 2 * SB, 128], f32,
                                          tag=f"ost{h}", name="ost")
                    st["i0"] = i0
                for c in range(nch):
                    nc.vector.tensor_scalar_mul(
                        st["tile"][:, st["filled"] + c, :],
                        o_t[:, slot(c) + 1:slot(c) + 129],
                        rec[:, c, :])
                st["filled"] += nch
                if st["filled"] >= 2 * SB:
                    flush_out(h)

            def emit_groups(h, I):
                sbp = plan[I]
                qT, kT = hstate[h]["qT"], hstate[h]["kT"]
                ptiles = []
                for group in sbp["groups"]:
                    if not group["units"]:
                        continue
                    s_t = s_ps.tile([128, 1024], f32, tag="s")
                    p_t = psb.tile([128, 1024], bf16, tag="p")
                    for (u, j, i, n) in group["runs"]:
                        nc.tensor.matmul(
                            s_t[:, u * QB:(u + n) * QB],
                            kT[:, j * QB:(j + 1) * QB],
                            qT[:, i * QB:(i + n) * QB],
                            start=True, stop=True)
                    gw = group["gw_units"] * QB
                    nc.scalar.activation(p_t[:, 0:gw], s_t[:, 0:gw],
                                         AF.Exp, scale=SCALE)
                    ptiles.append(p_t)
                # masks (gpsimd), after exp
                for m in sbp["masks"]:
                    kind, g, u = m[0], m[1], m[2]
                    p_t = ptiles[g]
                    sl = p_t[:, u * QB:(u + 1) * QB]
                    if kind == "tri":
                        # keep q >= k: iota = -p + c >= 0
                        nc.gpsimd.affine_select(
                            out=sl, in_=sl,
                            compare_op=mybir.AluOpType.is_ge, fill=0.0,
                            base=0, pattern=[[1, QB]],
                            channel_multiplier=-1)
                    elif kind == "rows":
                        c0, rb = m[3], m[4]
                        sl2 = p_t[:, u * QB + c0:(u + 1) * QB]
                        # keep k-rows >= rb: iota = p - rb >= 0
                        nc.gpsimd.affine_select(
                            out=sl2, in_=sl2,
                            compare_op=mybir.AluOpType.is_ge, fill=0.0,
                            base=-rb, pattern=[[0, QB - c0]],
                            channel_multiplier=1)
                    else:  # "zero"
                        c0 = m[3]
                        nc.gpsimd.memset(p_t[:, u * QB + c0:(u + 1) * QB],
                                         0.0)
                return ptiles

            # ---- main loop: heads interleaved at superblock granularity,
            # software-pipelined by one superblock per head (emit S^T+exp+
            # masks for (h, I), then PV+finalize for (h, I-1)).
            for I, sbp in enumerate(plan):
                for h in range(H_PER_CORE):
                    emit_prep(h, sbp["i0"] + sbp["n_chunks"])
                    ptiles = emit_groups(h, I)
                    if hstate[h]["pending"] is not None:
                        emit_pv_finalize(h, hstate[h]["pending"])
                    hstate[h]["pending"] = (I, ptiles)
            for h in range(H_PER_CORE):
                emit_pv_finalize(h, hstate[h]["pending"])
                flush_out(h)

    nc.compile()
    return nc


def _run(query, key, value, cu_seqlens, trace=False, **spmd_kwargs):
    from concourse import bass_utils

    query = np.ascontiguousarray(np.asarray(query, dtype=np.float32))
    key = np.ascontiguousarray(np.asarray(key, dtype=np.float32))
    value = np.ascontiguousarray(np.asarray(value, dtype=np.float32))
    cu = np.asarray(cu_seqlens, dtype=np.int64)

    nc = _build(cu)
    in_maps = []
    for c in range(N_CORES):
        hs = slice(c * H_PER_CORE, (c + 1) * H_PER_CORE)
        in_maps.append({
            "q": np.ascontiguousarray(query[:, hs, :]),
            "k": np.ascontiguousarray(key[:, hs, :]),
            "v": np.ascontiguousarray(value[:, hs, :]),
        })
    res = bass_utils.run_bass_kernel_spmd(nc, in_maps, list(range(N_CORES)),
                                          trace=trace, **spmd_kwargs)
    out = np.empty((L, H, D), dtype=np.float32)
    for c in range(N_CORES):
        out[:, c * H_PER_CORE:(c + 1) * H_PER_CORE, :] = res.results[c]["out"]
    return out, res


def kernel(query, key, value, cu_seqlens):
    out, _ = _run(query, key, value, cu_seqlens)
    return out


# revision 25
# speedup vs baseline: 1.1970x; 1.1970x over previous
"""Varlen causal attention (flash_attn_varlen semantics) on 8 Trainium2 cores.

Sharding: 16 heads across 8 cores (2 heads/core, Ulysses-style head shard,
identity comms). Each core runs the same SPMD Bass program on its head slice.

Key design (transpose-free inner loop): compute S^T = K @ Q^T instead of
S = Q @ K^T.  Then P^T = exp(S^T * scale) comes out of the activation engine
already in [k, q] layout, which is exactly the stationary-operand layout the
PV matmul needs (lhsT = P^T chunk, rhs = V block) -- no P transposes at all.

Per head:
  prep: load Q,K,V (both heads fused per DMA, 1KB contiguous elements, Q/K
        issued before V); PE-transpose Q,K into [D, L] bf16; V + ones col.
  main: for each 256-row q superblock, for each in-mask k block j:
        S^T tile = K_j @ Q^T  (bf16, PSUM f32), exp on ScalarE (bf16 out),
        causal/segment masking on GpSimd, then PV matmuls accumulate
        O[q, 0:130] per 128-q chunk over j (col 0 = softmax denominator from
        a ones column in V).  Finalize: reciprocal + scale on DVE, DMA out.
The (I, j) tile list, trimmed to the causal x segment block mask, is
specialized on the host from cu_seqlens at trace time.  Width-2 tiles are
packed first so no S^T matmul straddles a PSUM bank.
"""

import numpy as np

L = 4096
H = 16
D = 128
N_CORES = 8
H_PER_CORE = H // N_CORES
SCALE = 1.0 / float(np.sqrt(D))
QB = 128          # q/k block size
SB = 2            # q blocks per superblock (256 q rows)
GROUP_UNITS = 8   # 128-col units per S^T PSUM group tile ([128,1024] f32)
BANK_UNITS = 4    # 128-col units per PSUM bank


def _seg_starts(cu: np.ndarray) -> np.ndarray:
    """Per-token segment start, exactly mirroring the reference searchsorted."""
    tok = np.arange(L)
    seg = np.searchsorted(cu[1:-1], tok, side="right")
    starts = np.concatenate([[0], cu[1:-1]])
    return starts[seg]


def _build_plan(cu: np.ndarray):
    """Host-side specialization of the block-sparse attention pattern.

    Returns a list (one entry per superblock I) of dicts:
      groups: list of groups; each group has
              runs:  [(u, j, i, n)]  one S^T matmul per run (n units wide)
              units: [(u, j, i)]     per-128-col bookkeeping
      masks:  list of ("tri"|"rows"|"zero", group_idx, unit_off, *args)
      pv:     {chunk i: [(group_idx, unit_off, j), ...]}
    """
    ss = _seg_starts(cu)
    n_qb = L // QB
    k_lo_b = [int(ss[i * QB]) // QB for i in range(n_qb)]
    bounds = [int(b) for b in cu[1:-1] if 0 < int(b) < L]

    plan = []
    for I in range(n_qb // SB):
        i0, i1 = SB * I, SB * I + SB - 1
        tiles = []
        for j in range(k_lo_b[i0], i1 + 1):
            qsb = max(i0, j)
            qeb = qsb
            for i in range(qsb, i1 + 1):
                if k_lo_b[i] <= j:
                    qeb = i + 1
                else:
                    break
            if qeb > qsb:
                tiles.append((j, qsb, qeb - qsb))
        # wide tiles first: keeps 2-unit tiles bank-aligned (no splits)
        tiles.sort(key=lambda t: (-t[2], t[0]))

        groups = [{"runs": [], "units": []}]
        masks = []
        pv = {i: [] for i in range(i0, i1 + 1)}
        cursor = 0
        for (j, qsb, w) in tiles:
            rem, i = w, qsb
            while rem:
                if cursor == GROUP_UNITS:
                    groups.append({"runs": [], "units": []})
                    cursor = 0
                # runs may not straddle a PSUM bank boundary
                take = min(rem, BANK_UNITS - (cursor % BANK_UNITS))
                g = len(groups) - 1
                u = cursor
                groups[g]["runs"].append((u, j, i, take))
                for c in range(take):
                    ii = i + c
                    uu = u + c
                    groups[g]["units"].append((uu, j, ii))
                    pv[ii].append((g, uu, j))
                    if ii == j:
                        masks.append(("tri", g, uu))
                    q0u = ii * QB
                    for b in bounds:
                        if j * QB < b < (j + 1) * QB:
                            c0 = max(0, b - q0u)
                            rb = b - j * QB
                            if c0 < QB:
                                masks.append(("rows", g, uu, c0, rb))
                        elif (j + 1) * QB <= b:
                            c0 = b - q0u
                            if 0 <= c0 < QB:
                                masks.append(("zero", g, uu, c0))
                cursor += take
                i += take
                rem -= take
        # PV accumulation order per chunk must be deterministic; sort by j so
        # start/stop flags are simply first/last of the list.
        for i in pv:
            pv[i].sort(key=lambda t: t[2])
        plan.append({"groups": groups, "masks": masks, "pv": pv,
                     "i0": i0, "n_chunks": i1 - i0 + 1})
    return plan


def _build(cu: np.ndarray):
    import concourse.mybir as mybir
    import concourse.tile as tile
    from concourse import bacc
    from concourse.masks import make_identity

    f32 = mybir.dt.float32
    bf16 = mybir.dt.bfloat16
    AF = mybir.ActivationFunctionType
    n_qb = L // QB
    plan = _build_plan(cu)

    nc = bacc.Bacc("TRN2", target_bir_lowering=False, debug=False,
                   num_devices=N_CORES)
    q_d = nc.dram_tensor("q", [L, H_PER_CORE, D], f32, kind="ExternalInput")
    k_d = nc.dram_tensor("k", [L, H_PER_CORE, D], f32, kind="ExternalInput")
    v_d = nc.dram_tensor("v", [L, H_PER_CORE, D], f32, kind="ExternalInput")
    o_d = nc.dram_tensor("out", [L, H_PER_CORE, D], f32, kind="ExternalOutput")

    with tile.TileContext(nc) as tc:
        with (
            tc.tile_pool(name="consts", bufs=1) as consts,
            tc.tile_pool(name="stage", bufs=1) as stage,
            tc.tile_pool(name="big", bufs=1) as big,
            tc.tile_pool(name="psb", bufs=16) as psb,
            tc.tile_pool(name="osb", bufs=2) as osb,
            tc.tile_pool(name="rsb", bufs=4) as rsb,
            tc.tile_pool(name="s_ps", bufs=2, space="PSUM") as s_ps,
            tc.tile_pool(name="o_ps", bufs=2, space="PSUM") as o_ps,
            tc.tile_pool(name="tr_ps", bufs=2, space="PSUM") as tr_ps,
        ):
            ident = consts.tile([128, 128], f32)
            make_identity(nc, ident[:])
            # touch Exp once so ACT_TABLE_LOAD (~2.7us) happens during the
            # initial DMA wait instead of on the first real exp
            warm = consts.tile([128, 1], f32)
            nc.scalar.activation(warm[:], ident[:, 0:1], AF.Exp)

            # ---- DMA loads: both heads fused per span (contiguous 1KB per
            # (p, t) element => best HBM efficiency), early spans first ----
            qs = stage.tile([128, n_qb, H_PER_CORE, D], f32, tag="qs")
            ks = stage.tile([128, n_qb, H_PER_CORE, D], f32, tag="ks")
            vs = stage.tile([128, n_qb, H_PER_CORE, D], f32, tag="vs")
            spans = [(0, 4), (4, 4)] + [(b, 8) for b in range(8, n_qb, 8)]
            for b0, nb in spans:
                r = slice(b0 * QB, (b0 + nb) * QB)
                for t_d, t_s in ((qs, q_d), (ks, k_d), (vs, v_d)):
                    nc.sync.dma_start(
                        t_d[:, b0:b0 + nb, :, :],
                        t_s[r, :, :].rearrange("(t p) h d -> p t h d", p=128))

            # per-head prep state; transposes + V casts are emitted on demand
            # inside the main loop so compute tracks DMA arrival
            hstate = []
            for h in range(H_PER_CORE):
                vA = big.tile([128, n_qb, 130], bf16, tag=f"vA{h}")
                nc.gpsimd.memset(vA[:, :, 0:1], 1.0)
                qT = big.tile([128, L], bf16, tag=f"qT{h}")
                kT = big.tile([128, L], bf16, tag=f"kT{h}")
                hstate.append({"vA": vA, "qT": qT, "kT": kT, "done": 0,
                               "pending": None,
                               "ost": {"tile": None, "i0": 0, "filled": 0}})

            def emit_prep(h, need_b):
                hs = hstate[h]
                while hs["done"] < min(need_b, n_qb):
                    b0 = hs["done"]
                    for src, dstT in ((qs, hs["qT"]), (ks, hs["kT"])):
                        trp = tr_ps.tile([128, 4, 128], f32, tag="tr")
                        for t in range(4):
                            nc.tensor.transpose(trp[:, t, :],
                                                src[:, b0 + t, h, :],
                                                ident[:])
                        nc.vector.tensor_copy(
                            dstT[:, b0 * QB:(b0 + 4) * QB], trp[:, :, :])
                    nc.vector.tensor_copy(hs["vA"][:, b0:b0 + 4, 1:129],
                                          vs[:, b0:b0 + 4, h, :])
                    hs["done"] += 4

            def flush_out(h):
                st = hstate[h]["ost"]
                nf = st["filled"]
                if not nf:
                    return
                i0 = st["i0"]
                nc.sync.dma_start(
                    o_d[i0 * QB:(i0 + nf) * QB, h, :].rearrange(
                        "(t p) d -> p t d", p=128),
                    st["tile"][:, 0:nf, :])
                st["tile"] = None
                st["filled"] = 0

            def emit_pv_finalize(h, pend):
                # O chunk c lives at PSUM bank c//2, slot (c%2)*130; the two
                # accumulation groups per bank run sequentially (chunk-major)
                # as required by the one-group-per-bank rule.
                I, ptiles = pend
                sbp = plan[I]
                i0 = sbp["i0"]
                vA = hstate[h]["vA"]
                st = hstate[h]["ost"]
                nch = sbp["n_chunks"]
                slot = lambda c: c * 130
                o_t = o_ps.tile([128, 512], f32, tag="o")
                for c in range(nch):
                    i = i0 + c
                    lst = sbp["pv"][i]
                    for nn, (g, u, j) in enumerate(lst):
                        nc.tensor.matmul(
                            o_t[:, slot(c):slot(c) + 130],
                            ptiles[g][:, u * QB:(u + 1) * QB],
                            vA[:, j, 0:130],
                            start=(nn == 0), stop=(nn == len(lst) - 1))
                rec = rsb.tile([128, 2, 1], f32, tag="r")
                for b in range((nch + 1) // 2):
                    den = o_t[:, b * 512:b * 512 + 260].rearrange(
                        "p (c x) -> p c x", c=2)
                    nc.vector.reciprocal(rec[:, 2 * b:2 * b + 2, :],
                                         den[:, :, 0:1])
                if st["tile"] is None:
                    st["tile"] = osb.tile([128, 2 * SB, 128], f32,
                                          tag=f"ost{h}", name="ost")
                    st["i0"] = i0
                for c in range(nch):
                    nc.vector.tensor_scalar_mul(
                        st["tile"][:, st["filled"] + c, :],
                        o_t[:, slot(c) + 1:slot(c) + 129],
                        rec[:, c, :])
                st["filled"] += nch
                if st["filled"] >= 2 * SB:
                    flush_out(h)

            def emit_groups(h, I):
                sbp = plan[I]
                qT, kT = hstate[h]["qT"], hstate[h]["kT"]
                ptiles = []
                for group in sbp["groups"]:
                    if not group["units"]:
                        continue
                    s_t = s_ps.tile([128, 1024], f32, tag="s")
                    p_t = psb.tile([128, 1024], bf16, tag="p")
                    for (u, j, i, n) in group["runs"]:
                        nc.tensor.matmul(
                            s_t[:, u * QB:(u + n) * QB],
                            kT[:, j * QB:(j + 1) * QB],
                            qT[:, i * QB:(i + n) * QB],
                            start=True, stop=True)
                    gw = (group["units"][-1][0] + 1) * QB
                    nc.scalar.activation(p_t[:, 0:gw], s_t[:, 0:gw],
                                         AF.Exp, scale=SCALE)
                    ptiles.append(p_t)
                # masks (gpsimd), after exp
                for m in sbp["masks"]:
                    kind, g, u = m[0], m[1], m[2]
                    p_t = ptiles[g]
                    sl = p_t[:, u * QB:(u + 1) * QB]
                    if kind == "tri":
                        # keep q >= k: iota = -p + c >= 0
                        nc.gpsimd.affine_select(
                            out=sl, in_=sl,
                            compare_op=mybir.AluOpType.is_ge, fill=0.0,
                            base=0, pattern=[[1, QB]],
                            channel_multiplier=-1)
                    elif kind == "rows":
                        c0, rb = m[3], m[4]
                        sl2 = p_t[:, u * QB + c0:(u + 1) * QB]
                        # keep k-rows >= rb: iota = p - rb >= 0
                        nc.gpsimd.affine_select(
                            out=sl2, in_=sl2,
                            compare_op=mybir.AluOpType.is_ge, fill=0.0,
                            base=-rb, pattern=[[0, QB - c0]],
                            channel_multiplier=1)
                    else:  # "zero"
                        c0 = m[3]
                        nc.gpsimd.memset(p_t[:, u * QB + c0:(u + 1) * QB],
                                         0.0)
                return ptiles

            # ---- main loop: heads interleaved at superblock granularity,
            # software-pipelined by one superblock per head (emit S^T+exp+
            # masks for (h, I), then PV+finalize for (h, I-1)).  The last
            # head's two smallest superblocks are deferred to the end so the
            # final pipeline drain is short.
            pairs = [(I, h) for I in range(len(plan))
                     for h in range(H_PER_CORE)]
            lh = H_PER_CORE - 1
            if len(plan) > 4 and H_PER_CORE > 1:
                for p in ((0, lh), (1, lh)):
                    pairs.remove(p)
                    pairs.append(p)
            for (I, h) in pairs:
                sbp = plan[I]
                emit_prep(h, sbp["i0"] + sbp["n_chunks"])
                ptiles = emit_groups(h, I)
                if hstate[h]["pending"] is not None:
                    emit_pv_finalize(h, hstate[h]["pending"])
                hstate[h]["pending"] = (I, ptiles)
            for h in range(H_PER_CORE):
                emit_pv_finalize(h, hstate[h]["pending"])
                flush_out(h)

    nc.compile()
    return nc


def _run(query, key, value, cu_seqlens, trace=False, **spmd_kwargs):
    from concourse import bass_utils

    query = np.ascontiguousarray(np.asarray(query, dtype=np.float32))
    key = np.ascontiguousarray(np.asarray(key, dtype=np.float32))
    value = np.ascontiguousarray(np.asarray(value, dtype=np.float32))
    cu = np.asarray(cu_seqlens, dtype=np.int64)

    nc = _build(cu)
    in_maps = []
    for c in range(N_CORES):
        hs = slice(c * H_PER_CORE, (c + 1) * H_PER_CORE)
        in_maps.append({
            "q": np.ascontiguousarray(query[:, hs, :]),
            "k": np.ascontiguousarray(key[:, hs, :]),
            "v": np.ascontiguousarray(value[:, hs, :]),
        })
    res = bass_utils.run_bass_kernel_spmd(nc, in_maps, list(range(N_CORES)),
                                          trace=trace, **spmd_kwargs)
    out = np.empty((L, H, D), dtype=np.float32)
    for c in range(N_CORES):
        out[:, c * H_PER_CORE:(c + 1) * H_PER_CORE, :] = res.results[c]["out"]
    return out, res


def kernel(query, key, value, cu_seqlens):
    out, _ = _run(query, key, value, cu_seqlens)
    return out
